# revision 45
# baseline (speedup 1.0000x reference)
"""Trainium2 Bass kernel for nn_MultiHeadCrossAttention (B=4, N=2048, C=256, H=4, d=64).

Sharding: 8 cores, core c -> (batch b = c//2, query-half qh = c%2).
Each core computes full 4-head cross-attention for its 1024-query slice of
its batch, plus the residuals and output projection. No collectives; the
host slices/transposes/casts inputs per core and concatenates the outputs.

With gamma == 0 (as produced by setup_inputs), the LAM channel-attention
block is exactly the identity, so:
    out = (t2_grad + q + attn_out) @ Wproj + bproj

Design (v5, ~93us vs 117.7us bf16 baseline):
 - PE warm-up dummies burn the DMA-latency window so real matmuls start
   at 2.4 GHz (HAM clock gate); input DMAs split across both HWDGE
   queues (sync + scalar) in consumption order.
 - t1-side tensors (t1T, Wk, Wv) ship as fp8e4 in DoubleRow layout
   [128, cc(2), n]: the C=256 projections take ONE DoubleRow matmul
   each and the t1 DMA bytes halve.  t1 feeds only k and v, so the ~3%
   quantization washes out in attention averaging.  t2/Wq/Wp stay bf16
   (q and t2 ride the residual directly).
 - Attention runs as four query-sweeps (m-pair x 512-query block).  Per
   sweep the xo DoubleRow accumulators are only [80, 512] x 2 heads
   (2 PSUM banks), which frees THREE rotating 2-bank S tiles — a
   depth-3 S -> exp -> S-recycle pipeline that hides the ~2.2us
   chain latency (~0.5us of which is semaphore/queue-pop cost) behind
   a ~1us/chunk cadence.  Tags rotate on a global chunk counter so
   sweep boundaries never collide.
 - exp (8.4M elements/core; only ACT and DVE can read PSUM) alternates
   whole [128, 1024] chunks between ACT (exact exp, scaled by e^0.4703)
   and DVE (Schraudolph fp8e5m2: uint8 round(S*A+B) bits reinterpreted;
   +-11.6% sawtooth).  All tiles share one geometric-mean scale, so any
   engine split is numerically consistent; the sawtooth cancels in the
   softmax ratio and the residue scales with sqrt(sum p^2) ~ 4%.
   fp8e5m2's 4 bits/octave spans +-10.9 sigma — no reachable NaN/inf.
 - v tiles [128, ko(2), h(4), 80] fp8e4: col 0 of each head's 80-block
   is the ones column (softmax denominator rides the DoubleRow xo
   matmul as output row 0), cols 65-79 pad the ko stride to 16B.
 - per-sweep normalization (DVE reciprocal -> GpSimd partition
   broadcast -> DVE multiply) overlaps the next sweep, whose first
   chunks are hoisted onto ACT.
 - final projection is TRANSPOSED (oT[c, q], channels on partitions):
   every matmul streams a fat N=512, bias rides evacuation as a
   per-partition scalar, and the host un-transposes.  The xT @ Wp term
   is precomputed into SBUF partials during the (DMA-paced) projection
   phase; pass A (heads 0-1) overlaps the last sweep via the freed S
   banks; pass B (heads 2-3) accumulates into the same PSUM after
   normalization, then one fused DVE scalar_tensor_tensor per tile
   adds bias + partial before the store.
All engine ops keep in/out partition bases equal (DVE/ACT lanes are
partition-locked); cross-partition moves go through GpSimd broadcast.
"""

from contextlib import ExitStack

import numpy as np

import concourse.bass as bass
import concourse.mybir as mybir
import concourse.tile as tile
from concourse import bacc
from concourse.bass_utils import run_bass_kernel_spmd

B, N, C, H, D = 4, 2048, 256, 4, 64
NCORES = 8
Q = 1024  # queries per core
SCALE = float(D) ** -0.5
FP32 = mybir.dt.float32
BF16 = mybir.dt.bfloat16
I16 = mybir.dt.int16
AF = mybir.ActivationFunctionType
ALU = mybir.AluOpType

# Schraudolph fp8e5m2 exp: bitcast(uint8(round(x * EXPA8 + EXPB8)))
# ~= c * exp(x/8) with c = e^0.4703 (geometric mean for EXPB8 = 62.5).
# 4 mantissa steps per octave, so the u8 bit range spans +-10.9 sigma of
# S/8 (no reachable NaN/inf/zero; +-11.6% sawtooth that cancels in the
# softmax ratio, remaining error ~ saw * sqrt(sum p^2) ~ 0.4%).  ACT's
# exact-exp chunks use bias=0.4703 to sit on the same scale.
EXPA8 = float(4.0 / np.log(2.0)) * SCALE
EXPB8 = 62.5

_CACHE = {}


def build_nc():
    nc = bacc.Bacc("TRN2", target_bir_lowering=False, debug=False,
                   num_devices=NCORES)
    MDT = BF16

    FP8D = mybir.dt.float8e4
    # t1-side tensors ship as fp8e4 in DoubleRow layout [128, cc(2), n]:
    # the C=256 contraction then takes ONE DoubleRow matmul instead of
    # two K=128 passes, and the t1 DMA bytes halve.  t1 feeds only k and
    # v (no residual path), so the ~3% fp8 quantization washes out in
    # the softmax / attention averaging.
    t1T_d = nc.dram_tensor("t1T", [128, 2 * N], FP8D, kind="ExternalInput")
    t2T_d = nc.dram_tensor("t2T", [C, Q], MDT, kind="ExternalInput")
    wq_d = nc.dram_tensor("wq", [C, C], MDT, kind="ExternalInput")
    wk_d = nc.dram_tensor("wk", [128, 2 * C], FP8D, kind="ExternalInput")
    wv_d = nc.dram_tensor("wv", [128, 2 * C], FP8D, kind="ExternalInput")
    wp_d = nc.dram_tensor("wp", [C, C], MDT, kind="ExternalInput")
    bpT_d = nc.dram_tensor("bpT", [C, 1], FP32, kind="ExternalInput")
    # output is stored TRANSPOSED (channels x queries); the host undoes it
    out_d = nc.dram_tensor("out", [C, Q], FP32, kind="ExternalOutput")

    with tile.TileContext(nc) as tc, ExitStack() as ctx:
        const = ctx.enter_context(tc.tile_pool(name="const", bufs=1))
        acts = ctx.enter_context(tc.tile_pool(name="acts", bufs=1))

        # ---- PE warm-up: the HAM clock gate starts at 1.2 GHz and only
        # reaches 2.4 GHz after ~3.4us of sustained PE activity.  The
        # first real matmul cannot start before ~11.5us (framework
        # preamble + input DMA latency), so burn that window on dummy
        # matmuls over a zeroed tile to enter the kT phase at full clock.
        warm = const.tile([128, 512], MDT, name="warm", tag="warm")
        nc.vector.memset(warm[:], 0.0)
        with tc.tile_pool(name="warmps", bufs=1, space="PSUM") as wpool:
            wps = wpool.tile([128, 512], FP32, name="wps", tag="wps")
            for _ in range(6):
                nc.tensor.matmul(wps[:], lhsT=warm[:, 0:128], rhs=warm[:],
                                 start=True, stop=True)

        # ---- load inputs (critical path first: wk, then t1T chunks) ----
        # DMAs are spread across BOTH HWDGE queues (sync=SP, scalar=ACT);
        # a single queue serializes at ~600ns per 128x512 chunk and left
        # the PE idle until 12.7us.  cc=0 chunks ride SP, cc=1 rides ACT.
        dmae = [nc.sync, nc.scalar]
        w_sb = {}
        for name in ("wq", "wp"):
            w_sb[name] = [const.tile([128, C], MDT, name=f"{name}{cc}",
                                     tag=f"{name}{cc}") for cc in range(2)]
        for name, dram in (("wk", wk_d), ("wv", wv_d)):
            w_sb[name] = const.tile([128, 2 * C], FP8D, name=name, tag=name)

        t1T = acts.tile([128, 2 * N], FP8D, name="t1T", tag="t1T")
        t1d3 = t1T[:].rearrange("p (cc n) -> p cc n", cc=2)
        t2T = [acts.tile([128, Q], MDT, name=f"t2T{cc}", tag=f"t2T{cc}")
               for cc in range(2)]

        def load_t1(nn):
            dmae[nn % 2].dma_start(
                out=t1d3[:, :, nn * 512:(nn + 1) * 512],
                in_=t1T_d[:].rearrange("p (cc n) -> p cc n", cc=2)
                [:, :, nn * 512:(nn + 1) * 512])

        def load_t2(nn):
            for cc in range(2):
                dmae[cc].dma_start(
                    out=t2T[cc][:, nn * 512:(nn + 1) * 512],
                    in_=t2T_d[cc * 128:(cc + 1) * 128, nn * 512:(nn + 1) * 512])

        # queue order = consumption order: the first kT matmul needs only
        # wk + t1(0); wq/wv/wp are needed progressively later.
        dmae[0].dma_start(out=w_sb["wk"][:], in_=wk_d[:])
        load_t1(0)
        load_t1(1)
        load_t1(2)
        load_t1(3)
        load_t2(0)
        for cc in range(2):
            dmae[cc].dma_start(out=w_sb["wq"][cc][:],
                               in_=wq_d[cc * 128:(cc + 1) * 128, :])
        load_t2(1)
        dmae[1].dma_start(out=w_sb["wv"][:], in_=wv_d[:])
        for cc in range(2):
            dmae[cc].dma_start(out=w_sb["wp"][cc][:],
                               in_=wp_d[cc * 128:(cc + 1) * 128, :])

        # wp_h[h] row 0 multiplies xon[h] row 0 = den*recip ~= 1 in the
        # final projection and is kept zero to kill that row; the bias is
        # added per-partition during the transposed-output evacuation.
        wp_h = []
        for h in range(4):
            t = const.tile([65, C], MDT, name=f"wph{h}", tag=f"wph{h}")
            nc.gpsimd.memset(t[:], 0.0)
            dmae[h % 2].dma_start(out=t[1:65, :],
                                  in_=wp_d[h * 64:(h + 1) * 64, :])
            wp_h.append(t)
        bpT = []
        for ccq in range(2):
            t = const.tile([128, 1], FP32, name=f"bpT{ccq}", tag=f"bpT{ccq}")
            dmae[ccq].dma_start(out=t[:],
                                in_=bpT_d[ccq * 128:(ccq + 1) * 128, :])
            bpT.append(t)

        # ---- phase 1: projections kT, qT, v ----
        kT = [acts.tile([128, N], MDT, name=f"kT{m}", tag=f"kT{m}")
              for m in range(2)]
        qT = [acts.tile([128, Q], MDT, name=f"qT{m}", tag=f"qT{m}")
              for m in range(2)]
        # fp8 DoubleRow v tiles, one per 256-key super-chunk: layout
        # [128p, ko(2), h(4), 80] where ko indexes the two 128-key
        # sub-chunks contracted together, col 0 of each head's 80-block
        # is the softmax-denominator "ones" column, cols 1-64 hold v and
        # 65-79 are zero pad (DoubleRow needs the ko stride 16B-aligned).
        FP8 = mybir.dt.float8e4
        FP8R = mybir.dt.float8e5
        U8 = mybir.dt.uint8
        v_sb = []
        for sc in range(8):
            t = acts.tile([128, 2 * 4 * 80], FP8, name=f"v{sc}", tag=f"v{sc}")
            nc.gpsimd.memset(t[:], 0.0)
            v4 = t[:].rearrange("p (ko h e) -> p ko h e", ko=2, h=4)
            for ko in range(2):
                nc.gpsimd.memset(v4[:, ko, :, 0:1], 1.0)
            v_sb.append(t)

        # attention pools (opened before projections so the first S/exp
        # pairs can be hoisted into the projection phase)
        xT = [acts.tile([128, Q], MDT, name=f"xT{m}", tag=f"xT{m}")
              for m in range(2)]
        # SBUF partials holding the transposed xT @ Wp projection term
        opart = [[acts.tile([128, 512], FP32, name=f"op{half}{j}",
                            tag=f"op{half}{j}") for j in range(2)]
                 for half in range(2)]
        # normalized attention outputs, one [65, Q] tile per head; row 0
        # holds den*recip ~= 1 and is killed by the zero row in wp_h
        xon = [acts.tile([65, Q], MDT, name=f"xon{h}", tag=f"xon{h}")
               for h in range(4)]
        attn_ctx = ExitStack()
        spool = attn_ctx.enter_context(
            tc.tile_pool(name="spsum", bufs=1, space="PSUM"))
        ppool2 = ctx.enter_context(tc.tile_pool(name="pexp", bufs=3))
        npool = ctx.enter_context(tc.tile_pool(name="norm", bufs=2))
        # exp outputs live in fp8 super tiles [128p, ko(2), q(1024)], one
        # per (j-block, super-chunk); the two ko slices are written by
        # the exps of consecutive key chunks and contracted together by
        # the DoubleRow xo matmul.
        sup_map = {}
        emitted = {(m, j): 0 for m in range(2) for j in range(2)}
        sweep_idx = {(0, 0): 0, (0, 1): 1, (1, 0): 2, (1, 1): 3}
        # ACT's exact exp is scaled to match the DVE Schraudolph
        # output's geometric-mean scale c = e^0.4703 (for EXPB8 = 62.5);
        # mixed-scale terms would corrupt the softmax sum
        expshift5_sb = const.tile([128, 1], FP32, name="expshift5",
                                  tag="expshift5")
        nc.vector.memset(expshift5_sb[:], 0.4703)

        def emit_s_exp(m, j, kc, force_act=False):
            # one S tile [128, 2 heads x 512 q] per (key chunk, query
            # sweep j); three rotating 2-bank tags (enabled by the
            # sweep's small [80, 512] xo accumulators) give a depth-3 S
            # pipeline that hides the ~2.2us S -> exp -> S-recycle chain
            # behind the per-chunk engine cadence.
            sc, ko = kc // 2, kc % 2
            si = sweep_idx[(m, j)]
            if (m, j, sc) not in sup_map:
                sup_map[(m, j, sc)] = ppool2.tile(
                    [128, 2 * Q], FP8R, name="psup",
                    tag=f"psup{(8 * si + sc) % 3}")
            s3 = sup_map[(m, j, sc)][:].rearrange("p (ko q) -> p ko q",
                                                  ko=2)
            # rotate S tags on a GLOBAL chunk counter so a sweep's first
            # chunk never recycles the previous sweep's last-used tag
            gt = (16 * si + kc) % 3
            s_t = spool.tile([128, Q], FP32, name=f"sq{gt}",
                             tag=f"sq{gt}")
            for hh in range(2):
                base = hh * 64
                nc.tensor.matmul(
                    s_t[:, hh * 512:(hh + 1) * 512],
                    lhsT=kT[m][base:base + 64, kc * 128:(kc + 1) * 128],
                    rhs=qT[m][base:base + 64, j * 512:(j + 1) * 512],
                    start=True, stop=True)
            dst = s3[:, ko, :]
            # exp engines alternate whole chunks (GpSimd cannot read
            # PSUM); every tile is fp8e5m2 on a matched geometric scale
            # (ACT: exact exp * e^0.4703, DVE: Schraudolph), so any
            # assignment is numerically consistent.  force_act covers
            # chunks that overlap DVE's cast / normalization work.
            if force_act or kc % 2 == 0:
                nc.scalar.activation(dst, s_t[:], AF.Exp, scale=SCALE,
                                     bias=expshift5_sb[:])
            else:
                nc.vector.tensor_scalar(dst.bitcast(U8), s_t[:],
                                        EXPA8, EXPB8,
                                        op0=ALU.mult, op1=ALU.add)
            emitted[(m, j)] += 1

        # PSUM evacuation casts alternate DVE / ACT: both engines are
        # otherwise idle in this phase and each copy is ~0.5us.
        def evac(i, out, in_):
            if i % 3 != 2:
                nc.vector.tensor_copy(out, in_)
            else:
                nc.scalar.copy(out, in_)

        with tc.tile_pool(name="ppsum", bufs=2, space="PSUM") as ppool:
            wk3 = w_sb["wk"][:].rearrange("p (cc c) -> p cc c", cc=2)
            for m in range(2):
                for nn in range(N // 512):
                    ps = ppool.tile([128, 512], FP32, name="p", tag="p")
                    nc.tensor.matmul(
                        ps[:],
                        lhsT=wk3[:, :, m * 128:(m + 1) * 128],
                        rhs=t1d3[:, :, nn * 512:(nn + 1) * 512],
                        start=True, stop=True,
                        perf_mode=mybir.MatmulPerfMode.DoubleRow)
                    evac(m * 4 + nn, kT[m][:, nn * 512:(nn + 1) * 512],
                         ps[:])
            for m in range(2):
                for nn in range(Q // 512):
                    ps = ppool.tile([128, 512], FP32, name="p", tag="p")
                    for cc in range(2):
                        nc.tensor.matmul(
                            ps[:],
                            lhsT=w_sb["wq"][cc][:, m * 128:(m + 1) * 128],
                            rhs=t2T[cc][:, nn * 512:(nn + 1) * 512],
                            start=(cc == 0), stop=(cc == 1))
                    evac(m * 2 + nn, qT[m][:, nn * 512:(nn + 1) * 512],
                         ps[:])
            # the xT @ Wp partial of the final projection needs no
            # attention results: compute it here (transposed) into SBUF
            # partials while the engines are DMA-paced anyway
            for m in range(2):
                nc.gpsimd.tensor_add(xT[m][:], t2T[m][:], qT[m][:])
            for half in range(2):
                hsl = slice(half * 128, (half + 1) * 128)
                for j in range(2):
                    ps = ppool.tile([128, 512], FP32, name="p", tag="p")
                    for cc in range(2):
                        nc.tensor.matmul(
                            ps[:],
                            lhsT=w_sb["wp"][cc][:, hsl],
                            rhs=xT[cc][:, j * 512:(j + 1) * 512],
                            start=(cc == 0), stop=(cc == 1))
                    evac(half * 2 + j, opart[half][j][:], ps[:])
            for kc in range(4):
                emit_s_exp(0, 0, kc, force_act=True)
            wv3 = w_sb["wv"][:].rearrange("p (cc c) -> p cc c", cc=2)
            for sc in range(8):
                ps = ppool.tile([128, 512], FP32, name="p", tag="p")
                for ko in range(2):
                    kc = 2 * sc + ko
                    nc.tensor.matmul(
                        ps[:, ko * 256:(ko + 1) * 256],
                        lhsT=t1d3[:, :, kc * 128:(kc + 1) * 128],
                        rhs=wv3[:],
                        start=True, stop=True,
                        perf_mode=mybir.MatmulPerfMode.DoubleRow)
                v4 = v_sb[sc][:].rearrange("p (ko h e) -> p ko h e",
                                           ko=2, h=4)
                evac(sc, v4[:, :, :, 1:65],
                     ps[:].rearrange("p (ko h e) -> p ko h e", ko=2, h=4))

        def emit_xo(m, j, sc, xo_ps):
            # DoubleRow fp8 matmul: contracts 256 keys (2 ko sub-chunks)
            # per instruction, halving the xo stream count vs bf16 K=128.
            v4 = v_sb[sc][:].rearrange("p (ko h e) -> p ko h e", ko=2, h=4)
            s3 = sup_map[(m, j, sc)][:].rearrange("p (ko q) -> p ko q",
                                                  ko=2)
            for hh in range(2):
                h = 2 * m + hh
                nc.tensor.matmul(
                    xo_ps[hh][0:80, :],
                    lhsT=v4[:, :, h, :],
                    rhs=s3[:, :, hh * 512:(hh + 1) * 512],
                    start=(sc == 0), stop=(sc == 7),
                    perf_mode=mybir.MatmulPerfMode.DoubleRow)

        xopool = attn_ctx.enter_context(
            tc.tile_pool(name="xopsum", bufs=1, space="PSUM"))

        osb = ctx.enter_context(tc.tile_pool(name="osb", bufs=2))
        o_ps = None

        # sweep order: (m, j) = (0,0) (0,1) (1,0) (1,1).  Each sweep
        # runs 16 key chunks for one 512-query block of one head pair;
        # its normalization overlaps the next sweep's S/exp (whose first
        # chunks are hoisted ahead of the norm chain, on ACT since the
        # norm occupies DVE).
        sweeps = [(0, 0), (0, 1), (1, 0), (1, 1)]
        o_ps = None
        for si, (m, j) in enumerate(sweeps):
            xo_ps = [xopool.tile([80, 512], FP32, name=f"xo{hh}",
                                 tag=f"xo{hh}") for hh in range(2)]
            for sc in range(8):
                # stay one super-chunk of S/exp ahead of the DoubleRow
                # consumer so the exp engines chew chunk sc+1 while the
                # PE streams xo of chunk sc
                while emitted[(m, j)] < min(2 * sc + 4, 16):
                    emit_s_exp(m, j, emitted[(m, j)])
                emit_xo(m, j, sc, xo_ps)

            if si + 1 < 4:
                nm, nj = sweeps[si + 1]
                for kc in range(3):
                    emit_s_exp(nm, nj, kc, force_act=(kc < 2))
            else:
                # final-projection pass A, TRANSPOSED: accumulate
                # oT[c, q] = pair-0 head contributions in PSUM (tags
                # sq0/sq1, free once the last exps have read them).
                # Channels live on partitions, so every matmul streams a
                # fat N=512.  Runs on the PE while DVE/GpSimd normalize
                # the last sweep below; pass B accumulates heads 2-3
                # into the same PSUM after normalization.
                o_ps = [spool.tile([128, Q], FP32, name=f"oT{half}",
                                   tag=f"sq{half}") for half in range(2)]
                for h in range(2):
                    for half in range(2):
                        hsl = slice(half * 128, (half + 1) * 128)
                        for jj in range(Q // 512):
                            nc.tensor.matmul(
                                o_ps[half][:, jj * 512:(jj + 1) * 512],
                                lhsT=wp_h[h][:, hsl],
                                rhs=xon[h][:, jj * 512:(jj + 1) * 512],
                                start=(h == 0), stop=False)

            # normalize this sweep: row 0 of xo_ps[hh] = sum_k exp(S)
            jsl = slice(j * 512, (j + 1) * 512)
            for hh in range(2):
                recip = npool.tile([1, 512], FP32, name=f"recip{hh}",
                                   tag=f"recip{hh}")
                nc.vector.reciprocal_approx_fast(recip[:, :],
                                                 xo_ps[hh][0:1, :])
                bc_sb = npool.tile([65, 512], FP32, name=f"bc{hh}",
                                   tag=f"bc{hh}")
                nc.gpsimd.partition_broadcast(bc_sb[:], recip[:])
                nc.vector.tensor_mul(xon[2 * m + hh][:, jsl],
                                     xo_ps[hh][0:65, :], bc_sb[:])

        # final-projection pass B: accumulate pair-1 heads into the
        # transposed PSUM accumulators, then evacuate with the bias added
        # per-partition (DVE and ACT alternate) and store oT.
        for h in range(2, 4):
            for half in range(2):
                hsl = slice(half * 128, (half + 1) * 128)
                for j in range(Q // 512):
                    nc.tensor.matmul(
                        o_ps[half][:, j * 512:(j + 1) * 512],
                        lhsT=wp_h[h][:, hsl],
                        rhs=xon[h][:, j * 512:(j + 1) * 512],
                        start=False, stop=(h == 3))
        for j in range(Q // 512):
            for half in range(2):
                hsl = slice(half * 128, (half + 1) * 128)
                o_sb = osb.tile([128, 512], FP32, name="o", tag=f"o{j}")
                # out = (head_psum + bias) + xT@Wp partial, fused on DVE
                nc.vector.scalar_tensor_tensor(
                    o_sb[:], o_ps[half][:, j * 512:(j + 1) * 512],
                    bpT[half][:], opart[half][j][:],
                    op0=ALU.add, op1=ALU.add)
                dmae[(half + j) % 2].dma_start(
                    out=out_d[hsl, j * 512:(j + 1) * 512],
                    in_=o_sb[:])

        attn_ctx.close()

    nc.finalize()
    return nc


def _get_nc():
    if "nc" not in _CACHE:
        _CACHE["nc"] = build_nc()
    return _CACHE["nc"]


def _bf16(a):
    import ml_dtypes

    return np.ascontiguousarray(a.astype(ml_dtypes.bfloat16))


def _fp8dr(a):
    # [C, M] -> DoubleRow fp8e4 [128, 2, M]
    import ml_dtypes

    a = np.asarray(a, dtype=np.float32).reshape(2, 128, -1)
    return np.ascontiguousarray(
        a.transpose(1, 0, 2).astype(ml_dtypes.float8_e4m3fn))


def make_in_maps(t2_grad, t1, Wq, Wkv, Wproj, bproj):
    t2 = np.asarray(t2_grad, dtype=np.float32)
    t1 = np.asarray(t1, dtype=np.float32)
    wq = _bf16(np.asarray(Wq, dtype=np.float32))
    wk = _fp8dr(np.ascontiguousarray(Wkv[:, :C], dtype=np.float32))
    wv = _fp8dr(np.ascontiguousarray(Wkv[:, C:], dtype=np.float32))
    wp = _bf16(np.asarray(Wproj, dtype=np.float32))
    bpT = np.ascontiguousarray(
        np.asarray(bproj, dtype=np.float32).reshape(C, 1))
    in_maps = []
    for c in range(NCORES):
        b, qh = c // 2, c % 2
        in_maps.append({
            "t1T": _fp8dr(t1[b].T),
            "t2T": _bf16(t2[b].T[:, qh * Q:(qh + 1) * Q]),
            "wq": wq, "wk": wk, "wv": wv, "wp": wp, "bpT": bpT,
        })
    return in_maps


def kernel(t2_grad, t1, Wq, Wkv, Wproj, bproj, gamma, _trace=False,
           _use_fp32r=True):
    gamma = np.asarray(gamma)
    if float(np.abs(gamma).max()) != 0.0:
        # LAM block is only the identity for gamma == 0; fall back to a
        # host reference for the general case (not exercised by the
        # reference setup_inputs, which fixes gamma = 0).
        return _host_reference(t2_grad, t1, Wq, Wkv, Wproj, bproj, gamma)

    nc = _get_nc()
    in_maps = make_in_maps(t2_grad, t1, Wq, Wkv, Wproj, bproj)
    res = run_bass_kernel_spmd(nc, in_maps, list(range(NCORES)), trace=_trace)
    out = np.empty((B, N, C), dtype=np.float32)
    for c in range(NCORES):
        b, qh = c // 2, c % 2
        out[b, qh * Q:(qh + 1) * Q, :] = res.results[c]["out"].T
    if _trace:
        _CACHE["last_result"] = res
    return out


def _host_reference(t2_grad, t1, Wq, Wkv, Wproj, bproj, gamma):
    t2 = np.asarray(t2_grad, dtype=np.float64)
    t1 = np.asarray(t1, dtype=np.float64)
    Wq = np.asarray(Wq, dtype=np.float64)
    Wkv = np.asarray(Wkv, dtype=np.float64)
    Wproj = np.asarray(Wproj, dtype=np.float64)
    bproj = np.asarray(bproj, dtype=np.float64)
    g = float(np.asarray(gamma).reshape(-1)[0])
    q = (t2 @ Wq).reshape(B, N, H, D).transpose(0, 2, 1, 3)
    kv = (t1 @ Wkv).reshape(B, N, 2, H, D).transpose(2, 0, 3, 1, 4)
    k, v = kv[0], kv[1]
    s = np.einsum('bhnd,bhmd->bhnm', q, k) * SCALE
    s = s - s.max(axis=-1, keepdims=True)
    p = np.exp(s)
    p /= p.sum(axis=-1, keepdims=True)
    x = np.einsum('bhnm,bhmd->bhnd', p, v)
    xp = x.transpose(0, 3, 1, 2).reshape(B, D, H * N)
    energy = xp @ xp.transpose(0, 2, 1)
    energy = energy - energy.max(axis=-1, keepdims=True)
    att = np.exp(energy)
    att /= att.sum(axis=-1, keepdims=True)
    lam_out = (att @ xp).reshape(B, D, H, N)
    lam_out = g * lam_out + xp.reshape(B, D, H, N)
    x = lam_out.transpose(0, 2, 3, 1)
    xo = x.transpose(0, 2, 1, 3).reshape(B, N, C) \
        + q.transpose(0, 2, 1, 3).reshape(B, N, C)
    return ((t2 + xo) @ Wproj + bproj).astype(np.float32)

